# revision 2
# baseline (speedup 1.0000x reference)
"""Multi-head attention (no-transpose head reshape) on 8 trn2 cores.

Problem: B=2, S=2048, D=1024, H=16, DH=64.
  query = q @ Wq + bq  (same for k, v)
  dq = query.reshape(B, H, S, DH)   # NO transpose: head h uses rows
                                    # [128h, 128(h+1)) of query, reinterpreted
                                    # as a [2048, 64] matrix.
  out[b,h] = softmax(dq_h @ dk_h.T / 8) @ dv_h

Sharding: 32 independent (b, h) tasks; core c gets b=c//4 and the 4 heads
4*(c%4)..4*(c%4)+3, i.e. rows 512*(c%4)..+512 of batch b. No collectives.

Per-core kernel (matmuls in fp32r except AV which is bf16):
  - inputs arrive host-transposed (xT [1024, 512]) so the contraction dim
    (din) is on partitions.
  - Q/K projections computed transposed: XqT[dout, m] = sum_din Wq[din,dout]
    * xT[din, m]; stored [128, 8, 512] where partition = (bq%2)*64 + d2,
    free = (bq//2, h*128 + a); the head matrix slice is
    dq_h.T[d2, s2=16a+bq] = XqT[bq*64+d2, h*128+a].
  - V projection computed in normal orientation and evicted directly into
    AV-lhsT layout av_lhs[ak, h, bk, 0:64] = dv rows, with a ones column at
    index 64 so the AV matmul also produces softmax denominators.
  - scores computed transposed per (h, m-half H, k-group bk):
    S_T[k=16ak+bk, m=16aq+2po+H] via lhsT = XkT slice [64, 128] and
    rhs = XqT slice [64, 4, 128]; exp on ACT (scale=1/8) -> bf16 attnT;
    AV matmul accumulates outT_aug [65, 1024] over the 16 bk.
  - finish: PE-transpose [65,128] chunks -> [128, 65]; divide by col 64;
    strided DMA into out[h, s2, d2] (s2 = 16a + 2j + H).
"""

import numpy as np

B, S, D, NH = 2, 2048, 1024, 16
DH = 64
NCORES = 8
HPC = NH * B // NCORES      # heads per core = 4
ROWS = HPC * (S // NH)      # projection rows per core = 512
PO = D // 128               # 8 din/dout tiles

_BUILT = {}


def _build_nc():
    if "nc" in _BUILT:
        return _BUILT["nc"]

    import concourse.bass as bass
    import concourse.bacc as bacc
    import concourse.tile as tile
    from concourse import mybir
    from concourse.masks import make_identity
    from contextlib import ExitStack

    f32 = mybir.dt.float32
    f32r = mybir.dt.float32r
    bf16 = mybir.dt.bfloat16
    Exp = mybir.ActivationFunctionType.Exp

    nc = bacc.Bacc("TRN2", target_bir_lowering=False, debug=False)

    # ---- DRAM I/O (per core) ----
    qT = nc.dram_tensor("qT", [D, ROWS], f32, kind="ExternalInput")
    kT = nc.dram_tensor("kT", [D, ROWS], f32, kind="ExternalInput")
    vT = nc.dram_tensor("vT", [D, ROWS], f32, kind="ExternalInput")
    Wq = nc.dram_tensor("Wq", [D, D], f32, kind="ExternalInput")
    Wk = nc.dram_tensor("Wk", [D, D], f32, kind="ExternalInput")
    Wv = nc.dram_tensor("Wv", [D, D], f32, kind="ExternalInput")
    # bq/bk prepped host-side as [128, 8] (partition p, dout-tile po); bv as
    # [1, 1024], broadcast across partitions on device.
    bq = nc.dram_tensor("bq", [128, PO], f32, kind="ExternalInput")
    bk = nc.dram_tensor("bk", [128, PO], f32, kind="ExternalInput")
    bv = nc.dram_tensor("bv", [1, D], f32, kind="ExternalInput")
    out = nc.dram_tensor("out", [HPC, S, DH], f32, kind="ExternalOutput")
    # out viewed so row s2 = 16a + r: index [h, r, a, d]
    out_v = out.ap().rearrange("h (a r) d -> h r a d", r=16)

    with tile.TileContext(nc) as tc, ExitStack() as ctx:
        P = 128
        KB = S // P            # 16 k-groups (bk)

        consts = ctx.enter_context(tc.tile_pool(name="consts", bufs=1))
        inputs = ctx.enter_context(tc.tile_pool(name="inputs", bufs=1))
        proj = ctx.enter_context(tc.tile_pool(name="proj", bufs=1))

        # persistent SBUF (inputs typed f32r so fp32r matmuls accept them)
        xqT = inputs.tile([P, PO, ROWS], f32r, tag="xqT")
        xkT = inputs.tile([P, PO, ROWS], f32r, tag="xkT")
        xvT = inputs.tile([P, PO, ROWS], f32r, tag="xvT")
        XqT = proj.tile([P, PO, ROWS], f32r, tag="XqT")    # Q proj (transposed)
        XkT = proj.tile([P, PO, ROWS], f32r, tag="XkT")
        XkTs = proj.tile([P, PO, ROWS], f32r, tag="XkTs")  # partition-swapped
        av_lhs = proj.tile([P, HPC, KB, DH + 1], bf16, tag="av_lhs")

        bq_sb = consts.tile([P, PO], f32, tag="bq")
        bk_sb = consts.tile([P, PO], f32, tag="bk")
        bv_sb = consts.tile([P, D], f32, tag="bv")
        ident = consts.tile([P, P], f32, tag="ident")

        nc.sync.dma_start(out=bq_sb[:], in_=bq[:])
        nc.sync.dma_start(out=bk_sb[:], in_=bk[:])
        bv_ap = bv.ap()
        bv_bcast = bass.AP(tensor=bv_ap.tensor, offset=bv_ap.offset,
                           ap=[[0, P], [1, D]])
        nc.sync.dma_start(out=bv_sb[:], in_=bv_bcast)
        make_identity(nc, ident[:])
        nc.vector.memset(av_lhs[:, :, :, DH:DH + 1], 1.0)

        for t in range(PO):
            nc.sync.dma_start(out=xqT[:, t, :], in_=qT[t * P:(t + 1) * P, :].bitcast(f32r))
            nc.sync.dma_start(out=xkT[:, t, :], in_=kT[t * P:(t + 1) * P, :].bitcast(f32r))
            nc.sync.dma_start(out=xvT[:, t, :], in_=vT[t * P:(t + 1) * P, :].bitcast(f32r))

        # ---------------- projections ----------------
        with tc.tile_pool(name="wrows", bufs=1) as wrows, \
             tc.tile_pool(name="wv_rows", bufs=1) as wv_rows, \
             tc.tile_pool(name="pj_ps", bufs=3, space="PSUM") as pj_ps:

            # Q and K: transposed orientation.
            for (W_d, x_in, X_out, b_sb) in ((Wq, xqT, XqT, bq_sb),
                                             (Wk, xkT, XkT, bk_sb)):
                w_tiles = []
                for dint in range(PO):
                    w = wrows.tile([P, D], f32r, tag=f"w{dint}")
                    nc.sync.dma_start(out=w[:], in_=W_d[dint * P:(dint + 1) * P, :].bitcast(f32r))
                    w_tiles.append(w)
                for po in range(PO):
                    ps = pj_ps.tile([P, ROWS], f32, tag="pjps")
                    for dint in range(PO):
                        nc.tensor.matmul(ps[:],
                                         w_tiles[dint][:, po * P:(po + 1) * P],
                                         x_in[:, dint, :],
                                         start=(dint == 0), stop=(dint == PO - 1))
                    nc.vector.tensor_scalar_add(X_out[:, po, :], ps[:], b_sb[:, po:po + 1])

            # V: normal orientation, evicted into av_lhs (+bias, bf16).
            wv_tiles = []
            for dint in range(PO):
                w = wv_rows.tile([P, D], f32r, tag=f"wv{dint}")
                nc.sync.dma_start(out=w[:], in_=Wv[dint * P:(dint + 1) * P, :].bitcast(f32r))
                wv_tiles.append(w)
            for h in range(HPC):
                for dhalf in range(2):
                    ps = pj_ps.tile([P, ROWS], f32, tag="pjps")
                    for dint in range(PO):
                        nc.tensor.matmul(ps[:],
                                         xvT[:, dint, h * P:(h + 1) * P],
                                         wv_tiles[dint][:, dhalf * ROWS:(dhalf + 1) * ROWS],
                                         start=(dint == 0), stop=(dint == PO - 1))
                    # av_lhs[p, h, 8*dhalf + bkr, 0:64] = ps[p, bkr*64 + d] + bv
                    nc.vector.tensor_add(
                        av_lhs[:, h, dhalf * 8:(dhalf + 1) * 8, 0:DH],
                        ps[:],
                        bv_sb[:, dhalf * ROWS:(dhalf + 1) * ROWS])

        # partition-swapped XkT (for lhsT parity matching)
        nc.sync.dma_start(out=XkTs[0:64, :, :], in_=XkT[64:128, :, :])
        nc.sync.dma_start(out=XkTs[64:128, :, :], in_=XkT[0:64, :, :])

        # ---------------- attention ----------------
        with tc.tile_pool(name="sc_ps", bufs=2, space="PSUM") as sc_ps, \
             tc.tile_pool(name="av_ps", bufs=1, space="PSUM") as av_ps, \
             tc.tile_pool(name="tr_ps", bufs=2, space="PSUM") as tr_ps, \
             tc.tile_pool(name="attn", bufs=3) as attn_pool, \
             tc.tile_pool(name="fin", bufs=4) as fin_pool:

            MH = ROWS * 2      # m-half = 1024 columns

            for h in range(HPC):
                for Hh in range(2):
                    out_ps = av_ps.tile([DH + 1, MH], f32, tag="avps")
                    for bkk in range(KB):
                        ksrc = XkT if (bkk % 2) == Hh else XkTs
                        lhsT = ksrc[Hh * 64:Hh * 64 + 64, bkk // 2, h * P:(h + 1) * P]
                        s_ps = sc_ps.tile([P, MH], f32, tag="scps")
                        for j2 in range(2):
                            rhs = XqT[Hh * 64:Hh * 64 + 64, 4 * j2:4 * j2 + 4,
                                      h * P:(h + 1) * P]
                            nc.tensor.matmul(s_ps[:, j2 * ROWS:(j2 + 1) * ROWS],
                                             lhsT, rhs, start=True, stop=True)
                        at = attn_pool.tile([P, MH], bf16, tag="at")
                        nc.scalar.activation(at[:], s_ps[:], Exp, scale=0.125)
                        for j2 in range(2):
                            nc.tensor.matmul(out_ps[:, j2 * ROWS:(j2 + 1) * ROWS],
                                             av_lhs[:, h, bkk, :],
                                             at[:, j2 * ROWS:(j2 + 1) * ROWS],
                                             start=(bkk == 0), stop=(bkk == KB - 1))
                    # evict + finish
                    oT = fin_pool.tile([DH + 1, MH], f32, tag="oT")
                    nc.vector.tensor_copy(oT[:], out_ps[:])
                    for j in range(MH // P):
                        tp = tr_ps.tile([P, DH + 1], f32, tag="trps")
                        nc.tensor.transpose(tp[:], oT[:, j * P:(j + 1) * P],
                                            ident[0:DH + 1, 0:DH + 1])
                        rcp = fin_pool.tile([P, 1], f32, tag="rcp")
                        nc.vector.reciprocal(rcp[:], tp[:, DH:DH + 1])
                        o = fin_pool.tile([P, DH], f32, tag="o")
                        nc.vector.tensor_scalar_mul(o[:], tp[:, 0:DH], rcp[:])
                        nc.sync.dma_start(out=out_v[h, 2 * j + Hh, :, :], in_=o[:])

    nc.compile()
    _BUILT["nc"] = nc
    return nc


def kernel(q, k, v, Wq, bq, Wk, bk, Wv, bv):
    from concourse.bass_utils import run_bass_kernel_spmd

    nc = _build_nc()

    q = np.asarray(q, dtype=np.float32)
    k = np.asarray(k, dtype=np.float32)
    v = np.asarray(v, dtype=np.float32)
    Wq = np.ascontiguousarray(np.asarray(Wq, dtype=np.float32))
    Wk = np.ascontiguousarray(np.asarray(Wk, dtype=np.float32))
    Wv = np.ascontiguousarray(np.asarray(Wv, dtype=np.float32))
    bq_t = np.ascontiguousarray(np.asarray(bq, np.float32).reshape(PO, 128).T)
    bk_t = np.ascontiguousarray(np.asarray(bk, np.float32).reshape(PO, 128).T)
    bv_t = np.ascontiguousarray(np.asarray(bv, np.float32).reshape(1, D))

    in_maps = []
    for c in range(NCORES):
        b = c // (NCORES // B)
        r0 = (c % (NCORES // B)) * ROWS
        in_maps.append({
            "qT": np.ascontiguousarray(q[b, r0:r0 + ROWS, :].T),
            "kT": np.ascontiguousarray(k[b, r0:r0 + ROWS, :].T),
            "vT": np.ascontiguousarray(v[b, r0:r0 + ROWS, :].T),
            "Wq": Wq, "Wk": Wk, "Wv": Wv,
            "bq": bq_t, "bk": bk_t, "bv": bv_t,
        })

    res = run_bass_kernel_spmd(nc, in_maps, core_ids=list(range(NCORES)))

    outp = np.empty((B, NH, S, DH), dtype=np.float32)
    for c in range(NCORES):
        b = c // (NCORES // B)
        h0 = (c % (NCORES // B)) * HPC
        outp[b, h0:h0 + HPC] = res.results[c]["out"]
    return outp


# revision 24
# speedup vs baseline: 15527.7700x; 15527.7700x over previous
"""Multi-head attention (no-transpose head reshape) on 8 trn2 cores.

Problem: B=2, S=2048, D=1024, H=16, DH=64.
  query = q @ Wq + bq  (same for k, v)
  dq = query.reshape(B, H, S, DH)   # NO transpose: head h uses rows
                                    # [128h, 128(h+1)) of query, reinterpreted
                                    # as a [2048, 64] matrix.
  out[b,h] = softmax(dq_h @ dk_h.T / 8) @ dv_h

Sharding: 32 independent (b, h) tasks; core c gets b=c//4 and the 4 heads
4*(c%4)..4*(c%4)+3, i.e. rows 512*(c%4)..+512 of batch b. No collectives.

Per-core kernel (all matmuls bf16, fp32 PSUM accumulation):
  - inputs arrive host-transposed and bf16-cast (xT [1024, 512]) so the
    contraction dim (din) is on partitions.
  - Q/K projections computed transposed: XqT[dout, m] = sum_din Wq[din,dout]
    * xT[din, m]; stored [128, 8, 512] where partition = (bq%2)*64 + d2,
    free = (bq//2, h*128 + a); the head-matrix slice is
    dq_h.T[d2, s2=16a+bq] = XqT[bq*64+d2, h*128+a].
  - V projection computed in normal orientation and evicted directly into
    AV-lhsT layout av_lhs[ak, h, bk, 0:64] = dv rows, with a ones column at
    index 64 so the AV matmul also produces softmax denominators.
  - scores computed transposed per (h, m-half H, k-group bk):
    S_T[k=16ak+bk, m=16aq+2po+H] via lhsT = XkT slice [64, 128] (XkTs, the
    partition-swapped copy, when bk%2 != H) and rhs = XqT slice [64, 4, 128];
    exp on ACT (scale=1/8) -> bf16 attnT; AV matmul accumulates
    outT_aug [65, 1024] over the 16 bk (natural-parity bk first).
  - finish: PE-transpose [65,128] chunks -> [128, 65]; divide by col 64;
    one strided DMA per (h, H) into out[h, s2, d2] (s2 = 16a + 2j + H).

Timing notes (HW-profiled): fp32r matmuls trip a PE power throttle (half
clock) -- bf16 everywhere avoids it. One dma_start serializes on its queue,
so weights stream in quads and V's DMA is issued between Q's and K's so
V-projection fills the PE gap while K's weights land.
"""

import numpy as np

B, S, D, NH = 2, 2048, 1024, 16
DH = 64
NCORES = 8
HPC = NH * B // NCORES      # heads per core = 4
ROWS = HPC * (S // NH)      # projection rows per core = 512
PO = D // 128               # 8 din/dout tiles

_BUILT = {}


def _build_nc(reps=1, salt=0.0):
    if ("nc", reps, salt) in _BUILT:
        return _BUILT[("nc", reps, salt)]

    import concourse.bass as bass
    import concourse.bacc as bacc
    import concourse.tile as tile
    from concourse import mybir
    from concourse.masks import make_identity
    from contextlib import ExitStack

    f32 = mybir.dt.float32
    bf16 = mybir.dt.bfloat16
    Exp = mybir.ActivationFunctionType.Exp

    nc = bacc.Bacc("TRN2", target_bir_lowering=False, debug=False)

    qT = nc.dram_tensor("qT", [D, ROWS], bf16, kind="ExternalInput")
    kT = nc.dram_tensor("kT", [D, ROWS], bf16, kind="ExternalInput")
    vT = nc.dram_tensor("vT", [D, ROWS], bf16, kind="ExternalInput")
    Wq = nc.dram_tensor("Wq", [D, D], bf16, kind="ExternalInput")
    Wk = nc.dram_tensor("Wk", [D, D], bf16, kind="ExternalInput")
    Wv = nc.dram_tensor("Wv", [D, D], bf16, kind="ExternalInput")
    bq = nc.dram_tensor("bq", [128, PO], f32, kind="ExternalInput")
    bk = nc.dram_tensor("bk", [128, PO], f32, kind="ExternalInput")
    bv = nc.dram_tensor("bv", [1, D], f32, kind="ExternalInput")
    out = nc.dram_tensor("out", [HPC, S, DH], f32, kind="ExternalOutput")
    # out viewed so row s2 = 16a + r: index [h, a, r, d]
    out_w = out.ap().rearrange("h (a r) d -> h a r d", r=16)

    P = 128
    KB = S // P                # 16 k-groups (bk)
    MH = ROWS * 2              # m-half = 1024 columns

    with tile.TileContext(nc) as tc, ExitStack() as ctx:
        consts = ctx.enter_context(tc.tile_pool(name="consts", bufs=1))
        inputs_v = ctx.enter_context(tc.tile_pool(name="inputs_v", bufs=1))
        proj = ctx.enter_context(tc.tile_pool(name="proj", bufs=1))
        wrows = ctx.enter_context(tc.tile_pool(name="wrows", bufs=2))

        for _rep in range(reps):
            inputs_qk = tc.tile_pool(name="inputs_qk", bufs=1)
            iqk = inputs_qk.__enter__()

            xqT = iqk.tile([P, PO, ROWS], bf16, tag="xqT")
            xkT = iqk.tile([P, PO, ROWS], bf16, tag="xkT")
            xvT = inputs_v.tile([P, PO, ROWS], bf16, tag="xvT")
            XqT = proj.tile([P, PO, ROWS], bf16, tag="XqT")
            XkT = proj.tile([P, PO, ROWS], bf16, tag="XkT")
            XkTs = proj.tile([P, PO, ROWS], bf16, tag="XkTs")
            av_lhs = proj.tile([P, HPC, KB, DH + 1], bf16, tag="av_lhs")

            bq_sb = consts.tile([P, PO], f32, tag="bq")
            bk_sb = consts.tile([P, PO], f32, tag="bk")
            bv_sb = consts.tile([P, D], f32, tag="bv")
            ident = consts.tile([P, P], f32, tag="ident")

            nc.sync.dma_start(out=bq_sb[:], in_=bq[:])
            nc.sync.dma_start(out=bk_sb[:], in_=bk[:])
            bv_ap = bv.ap()
            bv_bcast = bass.AP(tensor=bv_ap.tensor, offset=bv_ap.offset,
                               ap=[[0, P], [1, D]])
            nc.sync.dma_start(out=bv_sb[:], in_=bv_bcast)
            make_identity(nc, ident[:])
            warm = consts.tile([1, 1], f32, tag="warm")
            nc.vector.memset(warm[:], salt)
            nc.scalar.activation(warm[:], warm[:], Exp, scale=1.0)
            nc.vector.memset(av_lhs[:, :, :, DH:DH + 1], 1.0)

            # ---------------- projections ----------------
            with tc.tile_pool(name="pj_ps", bufs=1, space="PSUM") as pj_ps:

                def qk_proj(W_d, x_src, x_in, X_out, b_sb, swp, pfx):
                    xsrc = x_src.ap().rearrange("(t p) c -> p t c", p=P)
                    for th in range(2):
                        nc.sync.dma_start(out=x_in[:, th * 4:(th + 1) * 4, :],
                                          in_=xsrc[:, th * 4:(th + 1) * 4, :])
                    wsrc = W_d.ap().rearrange("(t p) (quad c) -> p t quad c",
                                              p=P, quad=4)
                    wquads = []
                    for qd in range(4):
                        wq_ = wrows.tile([P, PO, D // 4], bf16,
                                         tag=f"wq{qd}", name=f"{pfx}wq{qd}")
                        nc.sync.dma_start(out=wq_[:], in_=wsrc[:, :, qd, :])
                        wquads.append(wq_)
                    # po-outer: each po completes early so evictions, swaps
                    # and the first scores pipeline with the remaining po's.
                    for po in range(PO):
                        ps = pj_ps.tile([P, ROWS], f32, tag=f"pq{po}",
                                        name=f"{pfx}pq{po}")
                        for dint in range(PO):
                            nc.tensor.matmul(
                                ps[:],
                                wquads[po // 2][:, dint, (po % 2) * P:(po % 2 + 1) * P],
                                x_in[:, dint, :],
                                start=(dint == 0), stop=(dint == PO - 1))
                        nc.vector.tensor_scalar_add(X_out[:, po, :], ps[:],
                                                    b_sb[:, po:po + 1])
                        if swp is not None:
                            nc.scalar.dma_start(out=swp[0:64, po, :],
                                                in_=X_out[64:128, po, :])
                            nc.scalar.dma_start(out=swp[64:128, po, :],
                                                in_=X_out[0:64, po, :])

                qk_proj(Wq, qT, xqT, XqT, bq_sb, None, "q")

                # V DMAs issued between Q's and K's: V-projection fills the
                # PE gap while K's weights stream in.
                wvsrc = Wv.ap().rearrange("(t p) (quad c) -> p t quad c",
                                          p=P, quad=4)
                wv_quads = []
                for qd in range(4):
                    wvq = wrows.tile([P, PO, D // 4], bf16,
                                     tag=f"wq{qd}", name=f"vwq{qd}")
                    nc.sync.dma_start(out=wvq[:], in_=wvsrc[:, :, qd, :])
                    wv_quads.append(wvq)
                xvsrc = vT.ap().rearrange("(t p) c -> p t c", p=P)
                for hf in range(2):
                    nc.sync.dma_start(out=xvT[:, hf * 4:(hf + 1) * 4, :],
                                      in_=xvsrc[:, hf * 4:(hf + 1) * 4, :])

                qk_proj(Wk, kT, xkT, XkT, bk_sb, XkTs, "k")

            inputs_qk.__exit__(None, None, None)

            # ------------- attention (V-proj interleaved) -------------
            with tc.tile_pool(name="sc_ps", bufs=2, space="PSUM") as sc_ps, \
                 tc.tile_pool(name="av_ps", bufs=1, space="PSUM") as av_ps, \
                 tc.tile_pool(name="pv_tr", bufs=2, space="PSUM") as pv_tr, \
                 tc.tile_pool(name="attn", bufs=24) as attn_pool, \
                 tc.tile_pool(name="fin", bufs=4) as fin_pool:

                # V projection (normal orientation) into av_lhs (+bias, bf16).
                for h in range(HPC):
                    for dhalf in range(2):
                        ps = pv_tr.tile([P, ROWS], f32, tag="pvtr")
                        for qh in range(2):
                            qd = dhalf * 2 + qh
                            for dint in range(PO):
                                nc.tensor.matmul(
                                    ps[:, qh * (ROWS // 2):(qh + 1) * (ROWS // 2)],
                                    xvT[:, dint, h * P:(h + 1) * P],
                                    wv_quads[qd][:, dint, :],
                                    start=(dint == 0), stop=(dint == PO - 1))
                        nc.vector.tensor_add(
                            av_lhs[:, h, dhalf * 8:(dhalf + 1) * 8, 0:DH],
                            ps[:],
                            bv_sb[:, dhalf * ROWS:(dhalf + 1) * ROWS])

                for h in range(HPC):
                    for Hh in range(2):
                        out_ps = av_ps.tile([DH + 1, MH], f32, tag="avps")
                        bk_order = ([b_ for b_ in range(KB) if b_ % 2 == Hh] +
                                    [b_ for b_ in range(KB) if b_ % 2 != Hh])
                        for bki, bkk in enumerate(bk_order):
                            ksrc = XkT if (bkk % 2) == Hh else XkTs
                            lhsT = ksrc[Hh * 64:Hh * 64 + 64, bkk // 2,
                                        h * P:(h + 1) * P]
                            s_ps = sc_ps.tile([P, MH], f32, tag="scps")
                            for j2 in range(2):
                                rhs = XqT[Hh * 64:Hh * 64 + 64,
                                          4 * j2:4 * j2 + 4, h * P:(h + 1) * P]
                                nc.tensor.matmul(
                                    s_ps[:, j2 * ROWS:(j2 + 1) * ROWS],
                                    lhsT, rhs, start=True, stop=True)
                            at = attn_pool.tile([P, MH], bf16, tag="at")
                            nc.scalar.activation(at[:], s_ps[:], Exp, scale=0.125)
                            for j2 in range(2):
                                nc.tensor.matmul(
                                    out_ps[:, j2 * ROWS:(j2 + 1) * ROWS],
                                    av_lhs[:, h, bkk, :],
                                    at[:, j2 * ROWS:(j2 + 1) * ROWS],
                                    start=(bki == 0), stop=(bki == KB - 1))
                        # evict + finish
                        oT = fin_pool.tile([DH + 1, MH], f32, tag="oT", bufs=2)
                        nc.vector.tensor_copy(oT[:], out_ps[:])
                        stage = fin_pool.tile([P, MH // P, DH], f32,
                                              tag="stage", bufs=2)
                        for j in range(MH // P):
                            tp = pv_tr.tile([P, DH + 1], f32, tag="pvtr")
                            nc.tensor.transpose(tp[:], oT[:, j * P:(j + 1) * P],
                                                ident[0:DH + 1, 0:DH + 1])
                            rcp = fin_pool.tile([P, 1], f32, tag="rcp")
                            nc.vector.reciprocal(rcp[:], tp[:, DH:DH + 1])
                            nc.vector.tensor_scalar_mul(stage[:, j, :],
                                                        tp[:, 0:DH], rcp[:])
                        nc.sync.dma_start(out=out_w[h, :, Hh::2, :], in_=stage[:])

    nc.compile()
    _dedupe_ldweights(nc)
    _BUILT[("nc", reps, salt)] = nc
    return nc


def _dedupe_ldweights(nc):
    """Remove InstLdweights that reload the stationary already resident in
    the PE array (consecutive matmuls sharing lhsT). Conservative: only
    sync-free duplicates; tracking resets at transposes (which clobber the
    array), drains, branches and any synced reload."""
    def key(a):
        return (str(a.memref), a.offset, str(a.ap), str(a.dtype))

    for f in nc.m.functions:
        for b in f.blocks:
            last = None
            keep = []
            for i in b.instructions:
                tn = type(i).__name__
                if tn == "InstLdweights":
                    k = key(i.ins[0])
                    si = i.sync_info
                    clean = (si is None) or (not si.on_wait and not si.on_update)
                    if last == k and clean:
                        continue
                    last = k
                elif tn == "InstMatmult":
                    if i.is_transpose:
                        last = None
                elif tn in ("InstDrain", "InstUnconditionalBranch", "InstCall"):
                    last = None
                keep.append(i)
            b.instructions[:] = keep


def kernel(q, k, v, Wq, bq, Wk, bk, Wv, bv):
    from concourse.bass_utils import run_bass_kernel_spmd
    import ml_dtypes

    nc = _build_nc()
    bfl = ml_dtypes.bfloat16

    q = np.asarray(q, dtype=np.float32)
    k = np.asarray(k, dtype=np.float32)
    v = np.asarray(v, dtype=np.float32)
    Wq_b = np.ascontiguousarray(np.asarray(Wq, np.float32).astype(bfl))
    Wk_b = np.ascontiguousarray(np.asarray(Wk, np.float32).astype(bfl))
    Wv_b = np.ascontiguousarray(np.asarray(Wv, np.float32).astype(bfl))
    bq_t = np.ascontiguousarray(np.asarray(bq, np.float32).reshape(PO, 128).T)
    bk_t = np.ascontiguousarray(np.asarray(bk, np.float32).reshape(PO, 128).T)
    bv_t = np.ascontiguousarray(np.asarray(bv, np.float32).reshape(1, D))

    in_maps = []
    for c in range(NCORES):
        b = c // (NCORES // B)
        r0 = (c % (NCORES // B)) * ROWS
        in_maps.append({
            "qT": np.ascontiguousarray(q[b, r0:r0 + ROWS, :].T.astype(bfl)),
            "kT": np.ascontiguousarray(k[b, r0:r0 + ROWS, :].T.astype(bfl)),
            "vT": np.ascontiguousarray(v[b, r0:r0 + ROWS, :].T.astype(bfl)),
            "Wq": Wq_b, "Wk": Wk_b, "Wv": Wv_b,
            "bq": bq_t, "bk": bk_t, "bv": bv_t,
        })

    res = run_bass_kernel_spmd(nc, in_maps, core_ids=list(range(NCORES)))

    outp = np.empty((B, NH, S, DH), dtype=np.float32)
    for c in range(NCORES):
        b = c // (NCORES // B)
        h0 = (c % (NCORES // B)) * HPC
        outp[b, h0:h0 + HPC] = res.results[c]["out"]
    return outp
